# revision 37
# baseline (speedup 1.0000x reference)
"""MixedScoreMultiHeadAttention on 8 TRN2 NeuronCores.

Sharding: data-parallel over batch B=8 (one batch element per core, no
collectives).  Per core (R=C=256, E=512, H=8, D=64, HID=128):

  1. QKV projections (bf16 matmuls; embeddings host-pretransposed to [E, S]).
  2. Per-head dot scores (K=64 matmuls, 2 heads packed via row groups);
     V projection deferred until after the first dot chunk so the
     channel-collapse DMA starts sooner.
  3. Channel-collapse via a DRAM bounce into S4 [32g+ch, pos] so the
     score-MLP runs channel-major with 4x tile_position row-packing (K=9).
     The bounce-out is split per 32-row group (4 DMA engines instead of 1)
     and each group's S4 gather chains on just its own bounce slice.  Cost
     rows of S4 are DMA'd straight from the input at load time.
  4. MLP waves (SW-pipelined at half-wave granularity): W1 runs in two
     [128,1024] PSUM half-tiles (bufs=2 -> true double buffering in 4
     banks), relu evict is split across ACT+DVE+Pool (3 engines), W2 is 4
     col-tiled M=8 matmuls, the mixed-score evict rotates over all three
     elementwise engines, and the DRAM-bounce scatter back to [r, (h, c)]
     logit tiles runs at quarter-rchunk granularity (bounce-out on the
     scalar HWDGE queue, gather-in on sync).
  5. Softmax without max-subtraction (logits are provably O(5)), mask
     applied multiplicatively after exp (fully-masked rows via +eps on the
     denominator), PE-transpose of the weights, AV producing out^T per
     r-half, final projection per r-half.  Softmax/AV for r-half 0 is
     spread one head per wave across the middle of the wave loop; r-half 1
     runs as a short tail after the last quarter-scatter.

The score-MLP weights are algebraically folded on the host:
  hidden = relu(concat_h[dot_h, alpha_h*cost] @ W1)
         = relu(sum_h dot_h * W1[2h,:] + cost * sum_h alpha_h W1[2h+1,:])
so the device sees a 9-channel input (8 raw-dot channels + 1 cost channel)
and an M9 [9, HID] matrix with the 1/sqrt(D) norm folded into the dot rows.
"""

import os

os.environ.setdefault("MYCRO_LOCAL_CACHE", "1")

import numpy as np
import ml_dtypes

import concourse.bass as bass
import concourse.mybir as mybir
import concourse.tile as tile
from concourse import bacc
from concourse.bass_utils import run_bass_kernel_spmd
from concourse.masks import make_identity

try:  # best-effort NTFF profiling hook (axon image lacks it by default)
    from antenv.axon_hooks import (
        get_axon_ntff_profile_hook,
        set_axon_ntff_profile_hook,
    )

    if get_axon_ntff_profile_hook() is None:
        from trn_agent_boot.trn_boot import _ntff_profile_via_ctypes

        set_axon_ntff_profile_hook(
            _ntff_profile_via_ctypes("/opt/axon/libaxon_pjrt.so")
        )
except Exception:
    pass

BF16 = mybir.dt.bfloat16
F32 = mybir.dt.float32
AF = mybir.ActivationFunctionType
ALU = mybir.AluOpType

B, R, C, E = 8, 256, 256, 512
H, D, HID = 8, 64, 128
NCORES = 8
NWAVES = 32  # 512 positions each: (2 r-rows per 32-row group) x 256 c

LAST_EXEC_NS = None
_CACHE = {}


def _build():
    nc = bacc.Bacc(
        "TRN2", target_bir_lowering=False, debug=False, enable_asserts=False
    )
    t = {}
    t["rembT"] = nc.dram_tensor("rembT", [E, R], BF16, kind="ExternalInput")
    t["cembT"] = nc.dram_tensor("cembT", [E, C], BF16, kind="ExternalInput")
    t["cost"] = nc.dram_tensor("cost16", [R, C], BF16, kind="ExternalInput")
    t["keep"] = nc.dram_tensor("keep16", [R, C], BF16, kind="ExternalInput")
    for w in ("wq", "wk", "wv", "wo"):
        t[w] = nc.dram_tensor(w, [E, E], BF16, kind="ExternalInput")
    t["m9"] = nc.dram_tensor("m9", [128, HID], BF16, kind="ExternalInput")
    t["w2"] = nc.dram_tensor("w2", [HID, H], BF16, kind="ExternalInput")
    t["out"] = nc.dram_tensor("out", [R, E], F32, kind="ExternalOutput")
    # DRAM bounce buffers for cross-partition reshapes (DMA cannot stride
    # the SBUF partition dim; DRAM APs are unconstrained)
    t["fb"] = nc.dram_tensor("fbounce", [2, H, 128, C], BF16, kind="Internal")
    # mixed-score bounce, position-major and 128-partition padded so the
    # per-wave bounce-out is ONE full-partition DMA and the gather-in reads
    # 4KB-contiguous (hh, c) runs per destination row
    t["mb"] = nc.dram_tensor(
        "mbounce", [2, 16, 2, 128, C], BF16, kind="Internal"
    )

    with tile.TileContext(nc) as tc:
        _kernel_body(tc, t)
    nc.compile()
    return nc


def _kernel_body(tc, t):
    nc = tc.nc
    with (
        tc.tile_pool(name="singles", bufs=1) as singles,
        tc.tile_pool(name="hp", bufs=3) as hpool,
        tc.tile_pool(name="pp", bufs=2) as ppool,
        tc.tile_pool(name="yp", bufs=2) as ypool,
        tc.tile_pool(name="dp", bufs=2) as dpool,
        tc.tile_pool(name="mmps", bufs=2, space="PSUM") as mmps,
        tc.tile_pool(name="w1ps", bufs=2, space="PSUM") as w1ps,
        tc.tile_pool(name="w2ps", bufs=2, space="PSUM") as w2ps,
    ):
        # ---- weights/constants to SBUF, split per chunk so compute can
        # start as soon as the first chunks land; wo is loaded last ----
        def wtile(name):
            return singles.tile([128, 4 * E], BF16, tag=name, name=name)

        wq_sb, wk_sb, wv_sb, wo_sb = map(wtile, ("wq", "wk", "wv", "wo"))
        remb_sb = singles.tile([128, 4 * R], BF16, tag="remb")
        cemb_sb = singles.tile([128, 4 * C], BF16, tag="cemb")

        def load_chunks(sb, th, n, eng=None):
            for k in range(4):
                (eng or nc.sync).dma_start(
                    out=sb[:, n * k : n * (k + 1)],
                    in_=th.ap()[128 * k : 128 * (k + 1), :],
                )

        # spread load issue across sync/scalar/gpsimd queues -- the HWDGE
        # dma_start occupies its sequencer ~1us each
        load_chunks(remb_sb, t["rembT"], R)
        load_chunks(wq_sb, t["wq"], E, nc.scalar)
        load_chunks(cemb_sb, t["cembT"], C)
        load_chunks(wk_sb, t["wk"], E, nc.gpsimd)
        load_chunks(wv_sb, t["wv"], E, nc.scalar)
        m9_sb = singles.tile([128, HID], BF16, tag="m9")
        nc.gpsimd.dma_start(out=m9_sb, in_=t["m9"].ap())
        w2_sb = singles.tile([HID, H], BF16, tag="w2")
        nc.gpsimd.dma_start(out=w2_sb, in_=t["w2"].ap())
        keep_sb = singles.tile([128, 2, C], BF16, tag="keep")
        nc.gpsimd.dma_start(
            out=keep_sb, in_=t["keep"].ap().rearrange("(i p) c -> p i c", p=128)
        )
        ident = singles.tile([128, 128], BF16, tag="ident")
        make_identity(nc, ident)

        # S4 [32g+ch, r''*256 + c]; cost channel (row 32g+8) comes straight
        # from the input tensor -- it does not depend on the dots.
        s4 = [
            singles.tile([128, 8192], BF16, tag=f"s4_{i}", name=f"s4_{i}")
            for i in range(2)
        ]
        for m in range(2):
            for g in range(4):
                nc.gpsimd.dma_start(
                    out=s4[m][32 * g + 8 : 32 * g + 9, :],
                    in_=t["cost"].ap()[
                        128 * m + 32 * g : 128 * m + 32 * (g + 1), :
                    ],
                )

        # ---- QKV projections (V deferred until after dot chunk 0) ----
        qt_sb = singles.tile([128, 4 * R], BF16, tag="qt")  # [hd, r]
        kt_sb = singles.tile([128, 4 * C], BF16, tag="kt")  # [hd, c]
        v_sb = singles.tile([128, 2 * E], BF16, tag="v")    # [c, hd]

        # ---- QKV + dot scores, j-interleaved: for each head-pair chunk j,
        # project Q and K then immediately take BOTH r-chunks' dots, so the
        # channel-collapse bounces for both halves launch as early as
        # possible.  V (only needed at AV time) runs after.  ----
        # S4[32g+ch, r''*256 + c] = feat_ch[128*m + 32*g + r'', c]
        f_sb = [
            singles.tile([128, 8 * C], BF16, tag=f"f{i}", name=f"f{i}")
            for i in range(2)
        ]
        for j in range(4):  # head-pair chunk
            ps = mmps.tile([128, 512], F32, tag="mm")
            for k in range(4):
                nc.tensor.matmul(
                    ps[:, 0:R],
                    lhsT=wq_sb[:, 512 * k + 128 * j : 512 * k + 128 * (j + 1)],
                    rhs=remb_sb[:, R * k : R * (k + 1)],
                    start=(k == 0), stop=(k == 3),
                )
            nc.scalar.copy(out=qt_sb[:, R * j : R * (j + 1)], in_=ps[:, 0:R])
            ps = mmps.tile([128, 512], F32, tag="mm")
            for k in range(4):
                nc.tensor.matmul(
                    ps[:, 0:C],
                    lhsT=wk_sb[:, 512 * k + 128 * j : 512 * k + 128 * (j + 1)],
                    rhs=cemb_sb[:, C * k : C * (k + 1)],
                    start=(k == 0), stop=(k == 3),
                )
            nc.vector.tensor_copy(out=kt_sb[:, C * j : C * (j + 1)], in_=ps[:, 0:C])
        # dots m-outer so rchunk 0's collapse DMA overlaps rchunk 1's dots
        # (launching both collapses at once saturates the DMA engines and
        # stalls the first waves)
        for m in range(2):
            for j in range(4):
                for s in range(2):
                    h = 2 * j + s
                    ps = mmps.tile([128, 256], F32, tag="mm")
                    nc.tensor.matmul(
                        ps,
                        lhsT=qt_sb[64 * s : 64 * (s + 1),
                                   R * j + 128 * m : R * j + 128 * (m + 1)],
                        rhs=kt_sb[64 * s : 64 * (s + 1), C * j : C * (j + 1)],
                        start=True, stop=True,
                        tile_position=(64 * s, 0),
                    )
                    if (j + s) % 2 == 0:
                        nc.scalar.copy(
                            out=f_sb[m][:, C * h : C * (h + 1)], in_=ps
                        )
                    else:
                        nc.vector.tensor_copy(
                            out=f_sb[m][:, C * h : C * (h + 1)], in_=ps
                        )
            if m == 0:
                for cc in range(2):
                    ps = mmps.tile([128, 512], F32, tag="mm")
                    for k in range(4):
                        nc.tensor.matmul(
                            ps,
                            lhsT=cemb_sb[:, C * k + 128 * cc :
                                         C * k + 128 * (cc + 1)],
                            rhs=wv_sb[:, 512 * k : 512 * (k + 1)],
                            start=(k == 0), stop=(k == 3),
                        )
                    nc.vector.tensor_copy(
                        out=v_sb[:, 512 * cc : 512 * (cc + 1)], in_=ps
                    )
            # channel-collapse bounce per 32-row group (4 DMA engines
            # working instead of 1); each group's gather chains on just its
            # own bounce slice.
            for g in range(4):
                nc.sync.dma_start(
                    out=t["fb"].ap()[m]
                    .transpose([1, 0, 2])[32 * g : 32 * (g + 1)],
                    in_=f_sb[m][32 * g : 32 * (g + 1), :].rearrange(
                        "p (ch c) -> p ch c", ch=8
                    ),
                )
            for g in range(4):
                nc.scalar.dma_start(
                    out=s4[m][32 * g : 32 * g + 8, :].rearrange(
                        "p (a b) -> p a b", a=32
                    ),
                    in_=t["fb"].ap()[m][:, 32 * g : 32 * (g + 1), :],
                )

        # ---- MLP waves (SW-pipelined) + interleaved softmax/AV/proj ----
        l_sb = [
            singles.tile([128, H * C], BF16, tag=f"l{i}", name=f"l{i}")
            for i in range(2)
        ]
        # mbig[32g+h', 512*np + 256*rp + c] = mixed for row (32g+2*np+rp), c
        mbig = singles.tile([128, 16 * 512], BF16, tag="mbig")
        pt_sb = [
            singles.tile([128, H * R], BF16, tag=f"pt{cc}", name=f"pt{cc}")
            for cc in range(2)
        ]
        ot_sb = singles.tile([128, 4 * R], BF16, tag="ot")  # [e, r]

        def gather_in(i, qq, final=False):
            # pull quarter qq of rchunk i from the bounce into l_sb; each
            # destination row reads one 4KB-contiguous (hh, c) run
            for g in range(4):
                src = t["mb"].ap()[i][4 * qq : 4 * (qq + 1), :,
                                      32 * g : 32 * g + 8, :]
                dst = l_sb[i][
                    32 * g + 8 * qq : 32 * g + 8 * (qq + 1), :
                ].rearrange("p (hh c) -> p hh c", hh=H)
                # split the last quarter's issue across two HWDGE queues to
                # shorten the serial tail before phase_c(1)
                eng = nc.scalar if (final and g % 2 == 1) else nc.sync
                eng.dma_start(out=dst, in_=src)

        def w2_one(n, g):
            w2p, h_sb = wave_state[n]
            nc.tensor.matmul(
                w2p[32 * g : 32 * g + 8, :],
                lhsT=w2_sb,
                rhs=h_sb[:, 512 * g : 512 * (g + 1)],
                start=True, stop=True,
                tile_position=(0, 32 * g),
            )

        def w2_finish(n):
            i, np_ = n // 16, n % 16
            w2p, _ = wave_state.pop(n)
            mst = mbig[:, 512 * np_ : 512 * (np_ + 1)]
            final = n == NWAVES - 1
            if final:
                # the last wave's evict+bounce gates phase_c(1): split the
                # evict across both engines and bounce via HWDGE (shorter
                # start-to-transfer latency than the Q7 SWDGE walk)
                nc.scalar.copy(out=mst[:, 0:256], in_=w2p[:, 0:256])
                nc.vector.tensor_copy(out=mst[:, 256:512], in_=w2p[:, 256:512])
                nc.sync.dma_start(
                    out=t["mb"].ap()[i][np_].transpose([1, 0, 2]),
                    in_=mst.rearrange("p (rp c) -> p rp c", rp=2),
                )
            else:
                if n % 2 == 0:
                    nc.vector.tensor_copy(out=mst, in_=w2p)
                else:
                    nc.scalar.copy(out=mst, in_=w2p)
                # per-wave bounce-out on the otherwise-idle Pool SWDGE
                # queue (sequencer cost ~25ns vs ~600ns on SP/ACT); rows
                # 32g+8..32g+32 are dead weight but keep it one run
                nc.gpsimd.dma_start(
                    out=t["mb"].ap()[i][np_].transpose([1, 0, 2]),
                    in_=mst.rearrange("p (rp c) -> p rp c", rp=2),
                )
            if np_ % 4 == 3:
                gather_in(i, np_ // 4, final=final)

        def phase_c_head_a(i, hh):
            # softmax head sub-step A: exp, masked sum, reciprocal
            p_f, sums, recips, pb = pc_state[i]
            hs = slice(C * hh, C * (hh + 1))
            nc.scalar.activation(out=p_f[:, hs], in_=l_sb[i][:, hs],
                                 func=AF.Exp)
            nc.vector.scalar_tensor_tensor(
                out=pb[:, hs],
                in0=p_f[:, hs],
                scalar=1.0,
                in1=keep_sb[:, i, :],
                op0=ALU.mult,
                op1=ALU.mult,
                accum_out=sums[:, hh : hh + 1],
            )
            nc.vector.tensor_scalar_add(
                out=sums[:, hh : hh + 1], in0=sums[:, hh : hh + 1],
                scalar1=1e-30,
            )
            nc.vector.reciprocal(
                out=recips[:, hh : hh + 1], in_=sums[:, hh : hh + 1]
            )

        def phase_c_head_b(i, hh):
            # sub-step B: PE transposes with the softmax normalization
            # folded in (identity scaled per-row by 1/rowsum -- the
            # transpose matmul pb^T @ diag(recip) both transposes AND
            # normalizes, killing a whole elementwise pass), and on odd
            # heads AV for the head pair.
            p_f, sums, recips, pb = pc_state[i]
            diag = dpool.tile([128, 128], BF16, tag="diag",
                              name=f"dg{i}_{hh}")
            # all-SBUF 16-bit op -> DVE 2x mode, ~130ns
            nc.vector.tensor_scalar_mul(
                out=diag, in0=ident, scalar1=recips[:, hh : hh + 1]
            )
            for cc in range(2):
                tp = mmps.tile([128, 128], F32, tag="mm",
                               name=f"tp{i}_{hh}_{cc}")
                # NOT nc.tensor.transpose: transpose-mode ignores the
                # identity operand's values, so the fold needs a real matmul
                nc.tensor.matmul(
                    tp,
                    lhsT=pb[:, C * hh + 128 * cc : C * hh + 128 * (cc + 1)],
                    rhs=diag,
                    start=True, stop=True,
                )
                dstp = pt_sb[cc][:, R * hh + 128 * i : R * hh + 128 * (i + 1)]
                if (hh + cc) % 2 == 0:
                    nc.scalar.copy(out=dstp, in_=tp)
                else:
                    nc.vector.tensor_copy(out=dstp, in_=tp)
            if hh % 2 == 1:
                # AV for head pair (hh-1, hh), r-half i
                j = hh // 2
                ps = mmps.tile([128, 128], F32, tag="mm", name=f"av{i}_{j}")
                for s in range(2):
                    h = 2 * j + s
                    for cc in range(2):
                        nc.tensor.matmul(
                            ps[64 * s : 64 * (s + 1), :],
                            lhsT=v_sb[:, 512 * cc + 64 * h :
                                      512 * cc + 64 * (h + 1)],
                            rhs=pt_sb[cc][:, R * h + 128 * i :
                                          R * h + 128 * (i + 1)],
                            start=(cc == 0), stop=(cc == 1),
                        )
                if j % 2 == 0:
                    nc.vector.tensor_copy(
                        out=ot_sb[:, R * j + 128 * i : R * j + 128 * (i + 1)],
                        in_=ps,
                    )
                else:
                    nc.scalar.copy(
                        out=ot_sb[:, R * j + 128 * i : R * j + 128 * (i + 1)],
                        in_=ps,
                    )
                if i in tail_state:
                    # incremental output projection: fold this head-pair's
                    # ot chunk into the accumulating proj as soon as it
                    # lands, so only the last chunk's matmul trails phase_c
                    tail_step(i, j)

        def phase_c_tail(i):
            # stage-burst form for the end tail: all exps, then all masked
            # sums, then all transpose/AV chains -- each engine runs its
            # stage back-to-back while the next engine pipelines behind it
            for hh in range(H):
                phase_c_head_a(i, hh)
            for hh in range(H):
                phase_c_head_b(i, hh)

        def pc_alloc(i):
            p_f = ppool.tile([128, H * C], F32, tag="p", name=f"p{i}")
            sums = singles.tile([128, H], F32, tag=f"sums{i}", name=f"sums{i}")
            recips = singles.tile(
                [128, H], F32, tag=f"recips{i}", name=f"recips{i}"
            )
            pb = singles.tile([128, H * C], BF16, tag=f"pb{i}", name=f"pb{i}")
            pc_state[i] = (p_f, sums, recips, pb)

        def tail_start(i):
            # proj accumulator reuses the w2ps rotation (free in the
            # epilogue once the matching wave's mixed evict has drained)
            tail_state[i] = w2ps.tile([128, 512], F32, tag="w2",
                                      name=f"yps{i}")

        def tail_step(i, k):
            nc.tensor.matmul(
                tail_state[i],
                lhsT=ot_sb[:, R * k + 128 * i : R * k + 128 * (i + 1)],
                rhs=wo_sb[:, 512 * k : 512 * (k + 1)],
                start=(k == 0), stop=(k == 3),
            )

        def tail_end(i):
            y = ypool.tile([128, 512], F32, tag="y", name=f"y{i}")
            nc.scalar.copy(out=y, in_=tail_state.pop(i))
            nc.sync.dma_start(
                out=t["out"].ap()[128 * i : 128 * (i + 1), 0:256],
                in_=y[:, 0:256],
            )
            nc.scalar.dma_start(
                out=t["out"].ap()[128 * i : 128 * (i + 1), 256:512],
                in_=y[:, 256:512],
            )

        def tail(i):
            # whole output projection for r-half i in one go
            tail_start(i)
            for k in range(4):
                tail_step(i, k)
            tail_end(i)

        load_chunks(wo_sb, t["wo"], E, nc.gpsimd)

        wave_state = {}
        pc_state = {}
        tail_state = {}
        pc_alloc(0)

        prev = None
        for n in range(NWAVES):
            i, np_ = n // 16, n % 16
            h_sb = hpool.tile([128, 2048], BF16, tag="h", name=f"h{n}")
            w2p = w2ps.tile([128, 512], F32, tag="w2", name=f"w2p{n}")
            wave_state[n] = (w2p, h_sb)
            # Burst issue: all 4 W1(n) back-to-back, then all 4 W2(n-1)
            # back-to-back.  Same-kind matmul bursts run ~2.5x faster per
            # instruction on TRN2 than interleaved kinds (measured 167 vs
            # 475 ns for the K=9 W1), and every matmul here is gated only
            # on work from iteration n-1 (W1 quarter-tile g is freed by
            # relu(n-1, g); W2(n-1) needs relu(n-1, 3)), so the PE never
            # blocks on this wave's own relus.  GPSIMD cannot touch PSUM,
            # so the relu evict alternates ACT/DVE per group.
            wtiles = []
            for g in range(4):
                wtiles.append(w1ps.tile(
                    [128, 512], F32, tag="w1", bufs=4, name=f"wps{n}_{g}"
                ))
                nc.tensor.matmul(
                    wtiles[g],
                    lhsT=m9_sb[32 * g : 32 * g + 9, :],
                    rhs=s4[i][32 * g : 32 * g + 9,
                              512 * np_ : 512 * (np_ + 1)],
                    start=True, stop=True,
                    tile_position=(32 * g, 0),
                )
            for g in range(4):
                if g % 2 == 0:
                    nc.scalar.activation(
                        out=h_sb[:, 512 * g : 512 * (g + 1)], in_=wtiles[g],
                        func=AF.Relu,
                    )
                else:
                    nc.vector.tensor_scalar_max(
                        out=h_sb[:, 512 * g : 512 * (g + 1)], in0=wtiles[g],
                        scalar1=0.0,
                    )
            if prev is not None:
                for g in range(4):
                    w2_one(prev, g)
                w2_finish(prev)
            prev = n
            if n == NWAVES - 1:
                # eager last wave: its W2 + evict + bounce gate phase_c(1),
                # so don't hold them for the epilogue
                for g in range(4):
                    w2_one(n, g)
                w2_finish(n)
                prev = None
            # spread r-half 0 softmax/AV in half-head steps through the
            # back half of the wave loop; its logits are complete after
            # wave 15's gather (+ DMA slack)
            if 16 <= n <= 31:
                hh, sub = (n - 16) // 2, (n - 16) % 2
                if sub == 0:
                    phase_c_head_a(0, hh)
                else:
                    phase_c_head_b(0, hh)
            if n == 25:
                pc_alloc(1)
        tail(0)
        tail_start(1)
        phase_c_tail(1)
        tail_end(1)


def _prep_inputs(row_emb, col_emb, cost_mat, attn_mask, Wq, Wk, Wv, Wo, W1,
                 W2, alpha):
    bf = ml_dtypes.bfloat16
    alpha_v = np.asarray(alpha, np.float32).reshape(-1)  # [H]
    W1 = np.asarray(W1, np.float32)
    # M9 row h (h<8): W1[2h,:]/sqrt(D); row 8: sum_h alpha_h * W1[2h+1,:]
    m9 = np.zeros((128, HID), np.float32)
    for g in range(4):
        for hh in range(H):
            m9[32 * g + hh] = W1[2 * hh] / np.sqrt(D)
        m9[32 * g + 8] = sum(alpha_v[hh] * W1[2 * hh + 1] for hh in range(H))
    shared = {
        "wq": np.asarray(Wq, np.float32).astype(bf),
        "wk": np.asarray(Wk, np.float32).astype(bf),
        "wv": np.asarray(Wv, np.float32).astype(bf),
        "wo": np.asarray(Wo, np.float32).astype(bf),
        "m9": m9.astype(bf),
        "w2": np.asarray(W2, np.float32).astype(bf),
    }
    in_maps = []
    for b in range(B):
        m = dict(shared)
        m["rembT"] = np.ascontiguousarray(
            np.asarray(row_emb[b], np.float32).T
        ).astype(bf)
        m["cembT"] = np.ascontiguousarray(
            np.asarray(col_emb[b], np.float32).T
        ).astype(bf)
        m["cost16"] = np.asarray(cost_mat[b, :, :, 0], np.float32).astype(bf)
        m["keep16"] = (~np.asarray(attn_mask[b])).astype(np.float32).astype(bf)
        in_maps.append(m)
    return in_maps


def kernel(**inputs) -> np.ndarray:
    global LAST_EXEC_NS
    if "nc" not in _CACHE:
        _CACHE["nc"] = _build()
    nc = _CACHE["nc"]
    in_maps = _prep_inputs(**inputs)
    trace = os.environ.get("KERNEL_TRACE", "0") == "1"
    res = run_bass_kernel_spmd(
        nc, in_maps, core_ids=list(range(NCORES)), trace=trace
    )
    LAST_EXEC_NS = res.exec_time_ns
    out = np.stack([np.asarray(res.results[b]["out"]) for b in range(B)])
    return out.astype(np.float32)
